# revision 6
# baseline (speedup 1.0000x reference)
"""Trainium2 Bass kernel: batched attention with softmax + fixed-key dropout.

reference:
    qk = einsum('bqd,bkd->bqk', q, k); s = qk / scale
    p = softmax(s, -1); keep = bernoulli(key(42), 0.2, p.shape)
    out = where(keep, p/0.2, 0) @ v

Sharding: B=16 batches split across 8 NeuronCores (2 per core). Each core
computes its own S x S attention slabs independently (no collectives).

Device algorithm per (batch, 128-row q-tile):
  1. PE: s = q^T k computed as 3 bf16 matmuls (hi/lo split per operand:
     qh*kh + ql*kh + qh*kl) accumulated in fp32 PSUM -> near-fp32 accuracy
     at full bf16 PE rate. Two PSUM halves of [128, S/2].
  2. DVE: row-max per half (negated), combined -> bias = -max * inv_scale.
  3. ACT: sexp = exp(s*inv_scale + bias) in fp16, accum_out -> Z (denom).
  4. GPSIMD: pm = sexp * dropout_mask (fp16 {0,1}, host-precomputed).
  5. pm^T via DMA xbar transpose (or PE identity-matmul transpose).
  6. PE: out_psum += pm^T.T @ v in fp16 over S.
  7. ACT: out = out_psum * (5/Z)  (dropout inverse scale folded in).
"""

import os
import numpy as np


def _ensure_path():
    import sys
    try:
        import concourse  # noqa: F401
        return
    except ImportError:
        pass
    for p in ("/opt/trn_rl_repo", "/root/.axon_site/_ro/trn_rl_repo"):
        if os.path.isdir(p) and p not in sys.path:
            sys.path.insert(0, p)
    import concourse  # noqa: F401


B, S, D, NCORES = 16, 2048, 128, 8
BL = B // NCORES  # batches per core

_cache = {}


def _build_nc(bl, s, d, tmode="xbar"):
    """Build and compile the Bass program for one core handling `bl` batches."""
    _ensure_path()
    from contextlib import ExitStack
    from concourse import bacc, mybir, tile
    from concourse.masks import make_identity

    dt = mybir.dt
    F32, F16, BF16 = dt.float32, dt.float16, dt.bfloat16
    AF = mybir.ActivationFunctionType
    ALU = mybir.AluOpType
    AX = mybir.AxisListType

    NI = s // 128           # q row tiles
    NJ = s // 128           # j chunks for pv/transpose
    HALF = s // 2           # qk computed in 2 psum halves
    CW = min(512, HALF)     # qk matmul moving width
    NCH = HALF // CW

    nc = bacc.Bacc("TRN2", target_bir_lowering=False, debug=False)

    qh = nc.dram_tensor("qh", [bl, 128, s], BF16, kind="ExternalInput").ap()
    ql = nc.dram_tensor("ql", [bl, 128, s], BF16, kind="ExternalInput").ap()
    kh = nc.dram_tensor("kh", [bl, 128, s], BF16, kind="ExternalInput").ap()
    kl = nc.dram_tensor("kl", [bl, 128, s], BF16, kind="ExternalInput").ap()
    v16 = nc.dram_tensor("v16", [bl, 128, s], F16, kind="ExternalInput").ap()
    mask = nc.dram_tensor("mask16", [bl, NI, 128, s], F16, kind="ExternalInput").ap()
    invs = nc.dram_tensor("inv_scale", [128, 1], F32, kind="ExternalInput").ap()
    out = nc.dram_tensor("out", [bl, s, d], F32, kind="ExternalOutput").ap()

    with tile.TileContext(nc) as tc, ExitStack() as ctx:
        pool = lambda name, bufs, **kw: ctx.enter_context(
            tc.tile_pool(name=name, bufs=bufs, **kw))

        const_p = pool("const", 1)
        qk_p = pool("qkT", 1)
        v_p = pool("v", 1)
        mask_p = pool("mask", 3)
        sexp_p = pool("sexp", 2)
        pm_p = pool("pm", 2)
        pmt_p = pool("pmt", 2)
        small_p = pool("small", 2)
        osb_p = pool("osb", 2)
        ps_s_p = pool("ps_s", 3 if tmode == "xbar" else 2, space="PSUM")
        ps_o_p = pool("ps_o", 2, space="PSUM")
        if tmode != "xbar":
            ps_t_p = pool("ps_t", 2, space="PSUM")

        ident = const_p.tile([128, 128], F16)
        make_identity(nc, ident[:])
        invs_sb = const_p.tile([128, 1], F32)
        nc.sync.dma_start(invs_sb[:], invs)

        for b in range(bl):
            qh_sb = qk_p.tile([128, s], BF16, tag="qh")
            ql_sb = qk_p.tile([128, s], BF16, tag="ql")
            kh_sb = qk_p.tile([128, s], BF16, tag="kh")
            kl_sb = qk_p.tile([128, s], BF16, tag="kl")
            v_sb = v_p.tile([128, s], F16)
            nc.sync.dma_start(qh_sb[:], qh[b])
            nc.sync.dma_start(ql_sb[:], ql[b])
            nc.sync.dma_start(kh_sb[:], kh[b])
            nc.sync.dma_start(kl_sb[:], kl[b])
            nc.sync.dma_start(v_sb[:], v16[b])

            for i in range(NI):
                mask_sb = mask_p.tile([128, s], F16)
                nc.sync.dma_start(mask_sb[:], mask[b, i])

                negmax = small_p.tile([128, 2], F32, tag="negmax")
                zh = small_p.tile([128, 2], F32, tag="zh")
                bias = small_p.tile([128, 1], F32, tag="bias")
                negmf = small_p.tile([128, 1], F32, tag="negmf")
                zs = small_p.tile([128, 1], F32, tag="zs")
                z5 = small_p.tile([128, 1], F32, tag="z5")
                rz = small_p.tile([128, 1], F32, tag="rz")
                sexp = sexp_p.tile([128, s], F16)

                iq = slice(i * 128, (i + 1) * 128)
                halves = []
                for h in range(2):
                    ps_s = ps_s_p.tile([128, HALF], F32)
                    halves.append(ps_s)
                    for c in range(NCH):
                        jc = slice(h * HALF + c * CW, h * HALF + (c + 1) * CW)
                        oc = slice(c * CW, (c + 1) * CW)
                        nc.tensor.matmul(ps_s[:, oc], lhsT=qh_sb[:, iq],
                                         rhs=kh_sb[:, jc], start=True, stop=False)
                        nc.tensor.matmul(ps_s[:, oc], lhsT=ql_sb[:, iq],
                                         rhs=kh_sb[:, jc], start=False, stop=False)
                        nc.tensor.matmul(ps_s[:, oc], lhsT=qh_sb[:, iq],
                                         rhs=kl_sb[:, jc], start=False, stop=True)
                    nc.vector.tensor_reduce(
                        negmax[:, h:h + 1], ps_s[:], axis=AX.X, op=ALU.max,
                        negate=True)
                # bias = -max_full * inv_scale  (negmax holds -max per half)
                nc.vector.tensor_tensor(
                    negmf[:], negmax[:, 0:1], negmax[:, 1:2], op=ALU.min)
                nc.gpsimd.tensor_scalar_mul(bias[:], negmf[:], invs_sb[:])
                for h in range(2):
                    nc.scalar.activation(
                        sexp[:, h * HALF:(h + 1) * HALF], halves[h][:],
                        AF.Exp, bias=bias[:], scale=invs_sb[:],
                        accum_out=zh[:, h:h + 1])
                nc.vector.tensor_tensor(zs[:], zh[:, 0:1], zh[:, 1:2], op=ALU.add)
                nc.gpsimd.tensor_scalar_mul(z5[:], zs[:], 0.2)
                nc.vector.reciprocal(rz[:], z5[:])  # 5 / Z

                # dropout: multiply by host-precomputed fp16 {0,1} keep-mask
                pm = pm_p.tile([128, s], F16)
                nc.gpsimd.tensor_tensor(pm[:], sexp[:], mask_sb[:], op=ALU.mult)

                pmt = pmt_p.tile([128, s], F16)
                if tmode == "xbar":
                    nc.sync.dma_start(
                        pmt[:].rearrange("p (c q) -> p c q", q=128), pm[:],
                        transpose=True)
                else:
                    ntb = (NJ + 7) // 8
                    for tb in range(ntb):
                        nt8 = min(8, NJ - tb * 8)
                        ps_t = ps_t_p.tile([128, 128 * nt8], F16)
                        for t8 in range(nt8):
                            t = tb * 8 + t8
                            nc.tensor.transpose(
                                ps_t[:, t8 * 128:(t8 + 1) * 128],
                                pm[:, t * 128:(t + 1) * 128], ident[:])
                        dst = pmt[:, tb * 1024:tb * 1024 + 128 * nt8]
                        if tb % 2 == 0:
                            nc.vector.tensor_copy(dst, ps_t[:])
                        else:
                            nc.scalar.copy(dst, ps_t[:])

                ps_o = ps_o_p.tile([128, d], F32)
                for tj in range(NJ):
                    nc.tensor.matmul(
                        ps_o[:],
                        lhsT=pmt[:, tj * 128:(tj + 1) * 128],
                        rhs=v_sb[:, tj * 128:(tj + 1) * 128],
                        start=(tj == 0), stop=(tj == NJ - 1))
                osb = osb_p.tile([128, d], F32)
                nc.scalar.mul(osb[:], ps_o[:], rz[:])
                nc.sync.dma_start(out[b, i * 128:(i + 1) * 128, :], osb[:])

    nc.compile()
    return nc


def _get_nc(bl, s, d, tmode):
    key = ("nc", bl, s, d, tmode)
    if key not in _cache:
        _cache[key] = _build_nc(bl, s, d, tmode)
    return _cache[key]


def _dropout_mask():
    """Exactly reproduce the reference's fixed-key dropout keep-mask."""
    key = ("mask", B, S)
    if key not in _cache:
        import jax
        with jax.default_device(jax.devices("cpu")[0]):
            keep = jax.random.bernoulli(jax.random.key(42), 1.0 - 0.8, (B, S, S))
            keep = np.asarray(keep)
        _cache[key] = keep.astype(np.float16)
    return _cache[key]


def _split_bf16(x):
    import ml_dtypes
    hi = x.astype(ml_dtypes.bfloat16)
    lo = (x - hi.astype(np.float32)).astype(ml_dtypes.bfloat16)
    return np.ascontiguousarray(hi), np.ascontiguousarray(lo)


def _host_prep(q, k, v, scale_factor):
    nc_, bl, s, d = NCORES, BL, S, D
    ni = s // 128
    nj = s // 128
    q = np.ascontiguousarray(q, dtype=np.float32).reshape(nc_, bl, s, d)
    k = np.ascontiguousarray(k, dtype=np.float32).reshape(nc_, bl, s, d)
    v = np.ascontiguousarray(v, dtype=np.float32).reshape(nc_, bl, s, d)
    qT = np.ascontiguousarray(q.transpose(0, 1, 3, 2))  # [nc, bl, 128, s]
    kT = np.ascontiguousarray(k.transpose(0, 1, 3, 2))
    qhi, qlo = _split_bf16(qT)
    khi, klo = _split_bf16(kT)
    # v_sb layout: [jp, cj*128 + dd] = v[cj*128+jp, dd]
    v16 = np.ascontiguousarray(
        v.reshape(nc_, bl, nj, 128, d).transpose(0, 1, 3, 2, 4)
    ).reshape(nc_, bl, 128, nj * d).astype(np.float16)
    mask16 = _dropout_mask().reshape(nc_, bl, ni, 128, s)
    inv = np.full((128, 1), np.float32(1.0) / np.float32(scale_factor[0]),
                  dtype=np.float32)
    in_maps = []
    for c in range(nc_):
        in_maps.append({
            "qh": qhi[c], "ql": qlo[c], "kh": khi[c], "kl": klo[c],
            "v16": v16[c], "mask16": mask16[c], "inv_scale": inv,
        })
    return in_maps


def _run(in_maps, tmode="xbar", trace=False):
    _ensure_path()
    from concourse.bass_utils import run_bass_kernel_spmd
    nc = _get_nc(BL, S, D, tmode)
    res = run_bass_kernel_spmd(
        nc, in_maps, core_ids=list(range(NCORES)), trace=trace)
    outs = [r["out"] for r in res.results]
    full = np.concatenate(outs, axis=0).reshape(B, S, D).astype(np.float32)
    return full, res


def kernel(q, k, v, scale_factor):
    in_maps = _host_prep(q, k, v, scale_factor)
    out, _ = _run(in_maps, tmode=os.environ.get("ATTN_TMODE", "xbar"))
    return out


def profile_run(q, k, v, scale_factor):
    """Returns (out, exec_time_ns) using NTFF profiling when available."""
    in_maps = _host_prep(q, k, v, scale_factor)
    out, res = _run(in_maps, tmode=os.environ.get("ATTN_TMODE", "xbar"),
                    trace=True)
    return out, res.exec_time_ns
